# revision 1
# baseline (speedup 1.0000x reference)
"""Multi-head self-attention (softmax over query axis) on 8 TRN2 NeuronCores.

Sharding: core c -> (batch b = c // 4, head-group g = c % 4). Each head-group
owns 4 of the 16 heads (256 of the 1024 projected features). Each core computes
its batch's QKV projections for its 4 heads, the (column-softmax) attention,
and a partial output projection (row-parallel); the host sums the 4 partials
per batch and adds the output bias.

Device pipeline (bf16 matmuls, fp32 accumulation):
  - DMA-transpose q,k,v (bf16) into feature-major xT [1024, 2048] tiles,
    all transposes issued up front so the xbar stays ahead of the PE
  - qhT/khT/vhT [256, 2048] = WxT.T @ xT      (PSUM accum over 8 d-tiles)
  - vhT -> vh [2048, 256] via PE transpose
  - per head pair: scoresT[k, q] = khT.T @ qhT for the upper-triangle
    chunks only; the two heads' K=64 matmuls are row-tiled into one
    [128, 1024] PSUM tile so they run concurrently; one packed exp(s/8)
    per chunk-pair; 0/1 mask multiply on diagonal chunks; per-row sums
    via packed DVE reduces; r = 1/sum folded into vh' = vh * r;
    ctxT[e, q] += vh'.T @ E (col-tiled pairs); the all-masked k=2047 row
    (uniform 1/S) is added as a rank-1 K=1 matmul
  - partial out [2048, 1024] = ctxT.T @ WoT   (fp32 out, DMA'd back)
"""

import numpy as np
import ml_dtypes

import concourse.bass as bass
import concourse.mybir as mybir
import concourse.tile as tile
from concourse import bacc
from concourse.bass_utils import run_bass_kernel_spmd
from concourse.masks import make_identity

BF16 = ml_dtypes.bfloat16
B, S, D, H, HD = 2, 2048, 1024, 16, 64
NCORES = 8
GROUPS = 4          # head-groups (4 heads each)
EG = D // GROUPS    # 256 features per group
SCALE = 1.0 / np.sqrt(HD)  # 0.125

bf = mybir.dt.bfloat16
f32 = mybir.dt.float32
MULT = mybir.AluOpType.mult
EXP = mybir.ActivationFunctionType.Exp
IDENT = mybir.ActivationFunctionType.Identity


def _build_kernel(has_bias: bool):
    nc = bacc.Bacc(
        "TRN2",
        target_bir_lowering=False,
        debug=False,
        enable_asserts=False,
        num_devices=NCORES,
    )

    xq = nc.dram_tensor("xq", [S, D], bf, kind="ExternalInput").ap()
    xk = nc.dram_tensor("xk", [S, D], bf, kind="ExternalInput").ap()
    xv = nc.dram_tensor("xv", [S, D], bf, kind="ExternalInput").ap()
    wqt = nc.dram_tensor("wqt", [D, EG], bf, kind="ExternalInput").ap()
    wkt = nc.dram_tensor("wkt", [D, EG], bf, kind="ExternalInput").ap()
    wvt = nc.dram_tensor("wvt", [D, EG], bf, kind="ExternalInput").ap()
    wot = nc.dram_tensor("wot", [EG, D], bf, kind="ExternalInput").ap()
    if has_bias:
        bq_d = nc.dram_tensor("bq_s", [EG], f32, kind="ExternalInput").ap()
        bk_d = nc.dram_tensor("bk_s", [EG], f32, kind="ExternalInput").ap()
        bv_d = nc.dram_tensor("bv_s", [EG], f32, kind="ExternalInput").ap()
    out_d = nc.dram_tensor("out", [S, D], f32, kind="ExternalOutput").ap()

    with tile.TileContext(nc) as tc:
        with tc.tile_pool(name="persist", bufs=1) as P, \
             tc.tile_pool(name="xpool", bufs=8) as XP:
            # --- weights + input transposes, issued in consumption order ---
            wq_t, wk_t, wv_t = [], [], []
            xts = {}
            for nm, wdram, xdram, lst in (
                ("q", wqt, xq, wq_t), ("k", wkt, xk, wk_t), ("v", wvt, xv, wv_t)
            ):
                for d in range(8):
                    t = P.tile([128, EG], bf, name=f"w{nm}{d}", tag=f"w{nm}{d}")
                    nc.sync.dma_start(t, wdram[d * 128:(d + 1) * 128, :])
                    lst.append(t)
                for d in range(8):
                    xt = XP.tile([128, S], bf, name=f"x{nm}", tag=f"x{nm}")
                    nc.sync.dma_start_transpose(
                        out=xt, in_=xdram[:, d * 128:(d + 1) * 128]
                    )
                    xts[(nm, d)] = xt
            wo_t = []
            for dk in range(2):
                t = P.tile([128, D], bf, name=f"wo{dk}", tag=f"wo{dk}")
                nc.sync.dma_start(t, wot[dk * 128:(dk + 1) * 128, :])
                wo_t.append(t)

            bias_t = {}
            if has_bias:
                for nm, dram in (("bq", bq_d), ("bk", bk_d), ("bv", bv_d)):
                    for e in range(2):
                        t = P.tile([128, 1], f32, name=f"{nm}b{e}", tag=f"{nm}b{e}")
                        nc.sync.dma_start(
                            t, dram[e * 128:(e + 1) * 128].rearrange("(p o) -> p o", o=1)
                        )
                        bias_t[(nm, e)] = t

            # --- constants ---
            # 0/1 keep-masks for the 4 diagonal-chunk variants: keep where
            # q_local > 128*j + k_local
            masks = []
            for j in range(4):
                mj = P.tile([128, 512], bf, name=f"mask{j}", tag=f"mask{j}")
                nc.gpsimd.memset(mj, 1.0)
                nc.gpsimd.affine_select(
                    out=mj,
                    in_=mj,
                    compare_op=mybir.AluOpType.is_gt,
                    fill=0.0,
                    base=-128 * j,
                    pattern=[[1, 512]],
                    channel_multiplier=-1,
                )
                masks.append(mj)
            ident = P.tile([128, 128], bf, name="ident", tag="ident")
            make_identity(nc, ident)
            ones_row = P.tile([1, 512], bf, name="ones_row", tag="ones_row")
            nc.gpsimd.memset(ones_row, 1.0)

            # --- persistent activations ---
            qhT = [P.tile([128, S], bf, name=f"qhT{e}", tag=f"qhT{e}") for e in range(2)]
            khT = [P.tile([128, S], bf, name=f"khT{e}", tag=f"khT{e}") for e in range(2)]
            vhT = [P.tile([128, S], bf, name=f"vhT{e}", tag=f"vhT{e}") for e in range(2)]
            vh = [P.tile([128, EG], bf, name=f"vh{st}", tag=f"vh{st}") for st in range(16)]
            ctxT = [P.tile([128, S], bf, name=f"ctxT{e}", tag=f"ctxT{e}") for e in range(2)]
            vspec = P.tile([1, EG], bf, name="vspec", tag="vspec")

            # ---------------- projections ----------------
            with tc.tile_pool(name="projpsum", bufs=8, space="PSUM") as PP:
                def project(nm, w_tiles, outT, bias_key):
                    psums = [
                        [PP.tile([128, 512], f32, name=f"pj{e}{sc}", tag="pj")
                         for sc in range(4)]
                        for e in range(2)
                    ]
                    for d in range(8):
                        xt = xts[(nm, d)]
                        for e in range(2):
                            for sc in range(4):
                                nc.tensor.matmul(
                                    psums[e][sc],
                                    lhsT=w_tiles[d][:, e * 128:(e + 1) * 128],
                                    rhs=xt[:, sc * 512:(sc + 1) * 512],
                                    start=(d == 0),
                                    stop=(d == 7),
                                )
                    for e in range(2):
                        for sc in range(4):
                            dst = outT[e][:, sc * 512:(sc + 1) * 512]
                            if has_bias:
                                nc.scalar.activation(
                                    dst, psums[e][sc], IDENT,
                                    bias=bias_t[(bias_key, e)], scale=1.0,
                                )
                            else:
                                nc.scalar.copy(dst, psums[e][sc])

                project("q", wq_t, qhT, "bq")
                project("k", wk_t, khT, "bk")
                project("v", wv_t, vhT, "bv")

            # vhT [256, S] -> vh [S, 256] via PE transpose
            with tc.tile_pool(name="tppsum", bufs=3, space="PSUM") as TP:
                for st in range(16):
                    for e in range(2):
                        tp = TP.tile([128, 128], bf, name="tp", tag="tp")
                        nc.tensor.transpose(
                            tp, vhT[e][:, st * 128:(st + 1) * 128], ident
                        )
                        nc.vector.tensor_copy(vh[st][:, e * 128:(e + 1) * 128], tp)

            # special row: vh[2047, :] / S at partition 0
            nc.sync.dma_start(vspec, vh[15][127:128, :])
            nc.vector.tensor_scalar_mul(vspec, vspec, 1.0 / S)

            # ---------------- attention (per head pair) ----------------
            with tc.tile_pool(name="attnpsum", bufs=1, space="PSUM") as AP_, \
                 tc.tile_pool(name="esb", bufs=10) as EP, \
                 tc.tile_pool(name="smalls", bufs=4) as SP:
                for p in range(2):
                    ctxps = [
                        AP_.tile([128, 512], f32, name=f"ctx{p}{qc}", tag="ctx", bufs=4)
                        for qc in range(4)
                    ]
                    Eprev = None
                    vhp_prev = None

                    def emit_ctx(kt, E_, vhp_):
                        qd_ = kt // 4
                        for qc in range(qd_, 4):
                            for a in range(2):
                                nc.tensor.matmul(
                                    ctxps[qc][64 * a:64 * (a + 1), :],
                                    lhsT=vhp_[:, 64 * a:64 * (a + 1)],
                                    rhs=E_[qc][:, 512 * a:512 * (a + 1)],
                                    start=(kt == 0),
                                    stop=False,
                                    skip_group_check=True,
                                )

                    for kt in range(16):
                        qd = kt // 4
                        nch = 4 - qd
                        E = [None] * 4
                        acc = SP.tile([128, 2, 4], f32, name="acc", tag="acc")
                        for qc in range(qd, 4):
                            # both heads' scores into one 2-bank psum tile
                            # (row-tiled K=64 matmuls run concurrently)
                            sc_ps = AP_.tile(
                                [128, 1024], f32, name="scps", tag="sc", bufs=2
                            )
                            for a in range(2):
                                nc.tensor.matmul(
                                    sc_ps[:, 512 * a:512 * (a + 1)],
                                    lhsT=khT[p][64 * a:64 * (a + 1), kt * 128:(kt + 1) * 128],
                                    rhs=qhT[p][64 * a:64 * (a + 1), qc * 512:(qc + 1) * 512],
                                    start=True,
                                    stop=True,
                                )
                            et = EP.tile([128, 1024], bf, name="E", tag="E")
                            nc.scalar.activation(et, sc_ps, EXP, bias=0.0, scale=SCALE)
                            j = qc - qd
                            if qc == qd:
                                # mask both heads' diagonal halves (0/1 mult),
                                # row-sums ride the accumulator
                                for a in range(2):
                                    nc.vector.scalar_tensor_tensor(
                                        out=et[:, 512 * a:512 * (a + 1)],
                                        in0=et[:, 512 * a:512 * (a + 1)],
                                        scalar=1.0,
                                        in1=masks[kt % 4],
                                        op0=MULT,
                                        op1=MULT,
                                        accum_out=acc[:, a, 0:1],
                                    )
                            else:
                                nc.vector.reduce_sum(
                                    acc[:, :, j:j + 1],
                                    et.rearrange("p (a x) -> p a x", a=2),
                                    axis=mybir.AxisListType.X,
                                )
                            E[qc] = et
                        # row sums -> r, folded into vh'
                        ssum = SP.tile([128, 2], f32, name="ssum", tag="ssum")
                        nc.vector.reduce_sum(
                            ssum, acc[:, :, 0:nch], axis=mybir.AxisListType.X
                        )
                        if kt == 15:
                            # k=2047 row is fully masked (sum 0) — keep 1/sum
                            # finite; its uniform-1/S weights are added below
                            ssum2 = SP.tile([128, 2], f32, name="ssum2", tag="ssum2")
                            nc.vector.tensor_scalar_add(ssum2, ssum, 1.0e-30)
                            ssum = ssum2
                        rr = SP.tile([128, 2], f32, name="rr", tag="rr")
                        nc.vector.reciprocal(rr, ssum)
                        vhp = SP.tile([128, 128], bf, name="vhp", tag="vhp")
                        for a in range(2):
                            nc.vector.tensor_scalar_mul(
                                vhp[:, 64 * a:64 * (a + 1)],
                                vh[kt][:, 128 * p + 64 * a:128 * p + 64 * (a + 1)],
                                rr[:, a:a + 1],
                            )
                        # software pipeline: previous kt's ctx matmuls land
                        # after this kt's scores so the PE never waits on the
                        # exp/sum/reciprocal chain
                        if Eprev is not None:
                            emit_ctx(kt - 1, Eprev, vhp_prev)
                        Eprev, vhp_prev = E, vhp
                    emit_ctx(15, Eprev, vhp_prev)
                    # uniform contribution of the fully-masked k=S-1 row
                    for qc in range(4):
                        nc.tensor.matmul(
                            ctxps[qc],
                            lhsT=vspec[0:1, 128 * p:128 * (p + 1)],
                            rhs=ones_row,
                            start=False,
                            stop=True,
                            skip_group_check=True,
                        )
                        nc.vector.tensor_copy(ctxT[p][:, qc * 512:(qc + 1) * 512], ctxps[qc])

            # ---------------- output projection ----------------
            with tc.tile_pool(name="oppsum", bufs=4, space="PSUM") as OP, \
                 tc.tile_pool(name="osb", bufs=4) as OS:
                for st in range(16):
                    for oc in range(2):
                        ps = OP.tile([128, 512], f32, name="op_ps", tag="op")
                        for dk in range(2):
                            nc.tensor.matmul(
                                ps,
                                lhsT=ctxT[dk][:, st * 128:(st + 1) * 128],
                                rhs=wo_t[dk][:, oc * 512:(oc + 1) * 512],
                                start=(dk == 0),
                                stop=(dk == 1),
                            )
                        ob = OS.tile([128, 512], f32, name="ob", tag="ob")
                        if (st + oc) % 2 == 0:
                            nc.vector.tensor_copy(ob, ps)
                        else:
                            nc.scalar.copy(ob, ps)
                        nc.sync.dma_start(
                            out_d[st * 128:(st + 1) * 128, oc * 512:(oc + 1) * 512], ob
                        )

    nc.compile()
    return nc


_NC_CACHE = {}


def _get_nc(has_bias: bool):
    if has_bias not in _NC_CACHE:
        _NC_CACHE[has_bias] = _build_kernel(has_bias)
    return _NC_CACHE[has_bias]


def make_in_maps(q, k, v, Wq, bq, Wk, bk, Wv, bv, Wo, bo, has_bias):
    WqT = np.ascontiguousarray(Wq.T).astype(BF16)
    WkT = np.ascontiguousarray(Wk.T).astype(BF16)
    WvT = np.ascontiguousarray(Wv.T).astype(BF16)
    WoT = np.ascontiguousarray(Wo.T)
    qb = [np.ascontiguousarray(q[b_]).astype(BF16) for b_ in range(B)]
    kb = [np.ascontiguousarray(k[b_]).astype(BF16) for b_ in range(B)]
    vb = [np.ascontiguousarray(v[b_]).astype(BF16) for b_ in range(B)]
    in_maps = []
    for c in range(NCORES):
        b_, g = c // GROUPS, c % GROUPS
        sl = slice(g * EG, (g + 1) * EG)
        m = {
            "xq": qb[b_],
            "xk": kb[b_],
            "xv": vb[b_],
            "wqt": np.ascontiguousarray(WqT[:, sl]),
            "wkt": np.ascontiguousarray(WkT[:, sl]),
            "wvt": np.ascontiguousarray(WvT[:, sl]),
            "wot": np.ascontiguousarray(WoT[sl, :]).astype(BF16),
        }
        if has_bias:
            m["bq_s"] = np.ascontiguousarray(bq[sl]).astype(np.float32)
            m["bk_s"] = np.ascontiguousarray(bk[sl]).astype(np.float32)
            m["bv_s"] = np.ascontiguousarray(bv[sl]).astype(np.float32)
        in_maps.append(m)
    return in_maps


def gather(results, bo):
    out = np.zeros((B, S, D), np.float32)
    for b_ in range(B):
        acc = np.zeros((S, D), np.float32)
        for g in range(GROUPS):
            acc += results[b_ * GROUPS + g]["out"]
        out[b_] = acc + bo.astype(np.float32)[None, :]
    return out


def kernel(q, k, v, Wq, bq, Wk, bk, Wv, bv, Wo, bo, **run_kwargs):
    q, k, v = (np.asarray(x, np.float32) for x in (q, k, v))
    Wq, bq, Wk, bk, Wv, bv, Wo, bo = (
        np.asarray(x, np.float32) for x in (Wq, bq, Wk, bk, Wv, bv, Wo, bo)
    )
    has_bias = bool(
        max(np.abs(bq).max(), np.abs(bk).max(), np.abs(bv).max()) > 0
    )
    nc = _get_nc(has_bias)
    in_maps = make_in_maps(q, k, v, Wq, bq, Wk, bk, Wv, bv, Wo, bo, has_bias)
    res = run_bass_kernel_spmd(
        nc, in_maps, core_ids=list(range(NCORES)), **run_kwargs
    )
    out = gather(res.results, bo)
    if run_kwargs:
        return out, res
    return out



# revision 7
# speedup vs baseline: 1.0801x; 1.0801x over previous
"""Multi-head self-attention (softmax over query axis) on 8 TRN2 NeuronCores.

Sharding: core c -> (batch b = c // 4, head-group g = c % 4). Each head-group
owns 4 of the 16 heads (256 of the 1024 projected features). Each core computes
its batch's QKV projections for its 4 heads, the (column-softmax) attention,
and a partial output projection (row-parallel); the host sums the 4 partials
per batch and adds the output bias.

Device pipeline (bf16 matmuls, fp32 accumulation):
  - host pre-transposes q,k,v into feature-major xT [1024, 2048] bf16, so the
    device does cheap straight DMA loads (the old dma_start_transpose path was
    descriptor-bound and took ~100us)
  - qhT/khT/vhT [256, 2048] = WxT.T @ xT      (PSUM accum over 8 d-tiles)
  - vhT -> vh [2048, 256] via PE transpose
  - per head pair: scoresT[k, q] = khT.T @ qhT for the upper-triangle
    chunks only; the two heads' K=64 matmuls are row-tiled into one
    [128, 1024] PSUM tile so they run concurrently. Diagonal chunks get the
    causal mask applied ON THE PE: one extra matmul per head accumulates
    (-8e9*UT).T @ I into the mixed 128-col block, so exp() emits exact zeros
    there and no vector masking is needed. One packed exp(s/8) per
    chunk-pair; per-row sums via packed DVE reduces; r = 1/sum folded into
    vh' = vh * r; ctxT[e, q] += vh'.T @ E (col-tiled pairs); the all-masked
    k=2047 row (uniform 1/S) is added as a rank-1 K=1 matmul
  - partial out [2048, 1024] = ctxT.T @ WoT, bf16 partials DMA'd back;
    the host sums the 4 partials per batch in fp32 and adds the bias
"""

import numpy as np
import ml_dtypes

import concourse.bass as bass
import concourse.mybir as mybir
import concourse.tile as tile
from concourse import bacc
from concourse.bass_utils import run_bass_kernel_spmd
from concourse.masks import make_identity

BF16 = ml_dtypes.bfloat16
B, S, D, H, HD = 2, 2048, 1024, 16, 64
NCORES = 8
GROUPS = 4          # head-groups (4 heads each)
EG = D // GROUPS    # 256 features per group
SCALE = 1.0 / np.sqrt(HD)  # 0.125
MASKNEG = -8.0e9    # -8e9 * SCALE = -1e9 -> exp == 0

bf = mybir.dt.bfloat16
f32 = mybir.dt.float32
MULT = mybir.AluOpType.mult
ADD = mybir.AluOpType.add
EXP = mybir.ActivationFunctionType.Exp
IDENT = mybir.ActivationFunctionType.Identity


def _build_kernel(has_bias: bool):
    nc = bacc.Bacc(
        "TRN2",
        target_bir_lowering=False,
        debug=False,
        enable_asserts=False,
        num_devices=NCORES,
    )

    # feature-major inputs [1024, 2048] bf16 (host-transposed)
    xqt = nc.dram_tensor("xqt", [D, S], bf, kind="ExternalInput").ap()
    xkt = nc.dram_tensor("xkt", [D, S], bf, kind="ExternalInput").ap()
    xvt = nc.dram_tensor("xvt", [D, S], bf, kind="ExternalInput").ap()
    wqt = nc.dram_tensor("wqt", [D, EG], bf, kind="ExternalInput").ap()
    wkt = nc.dram_tensor("wkt", [D, EG], bf, kind="ExternalInput").ap()
    wvt = nc.dram_tensor("wvt", [D, EG], bf, kind="ExternalInput").ap()
    wot = nc.dram_tensor("wot", [EG, D], bf, kind="ExternalInput").ap()
    if has_bias:
        bq_d = nc.dram_tensor("bq_s", [EG], f32, kind="ExternalInput").ap()
        bk_d = nc.dram_tensor("bk_s", [EG], f32, kind="ExternalInput").ap()
        bv_d = nc.dram_tensor("bv_s", [EG], f32, kind="ExternalInput").ap()
    out_d = nc.dram_tensor("out", [S, D], bf, kind="ExternalOutput").ap()

    with tile.TileContext(nc) as tc:
        with tc.tile_pool(name="persist", bufs=1) as P, \
             tc.tile_pool(name="xpool", bufs=8) as XP:
            # --- weights + inputs, issued in consumption order ---
            wq_t, wk_t, wv_t = [], [], []
            xts = {}
            for nm, wdram, xdram, lst in (
                ("q", wqt, xqt, wq_t), ("k", wkt, xkt, wk_t), ("v", wvt, xvt, wv_t)
            ):
                for d in range(8):
                    t = P.tile([128, EG], bf, name=f"w{nm}{d}", tag=f"w{nm}{d}")
                    nc.sync.dma_start(t, wdram[d * 128:(d + 1) * 128, :])
                    lst.append(t)
                for d in range(8):
                    xt = XP.tile([128, S], bf, name=f"x{nm}", tag=f"x{nm}")
                    nc.sync.dma_start(xt, xdram[d * 128:(d + 1) * 128, :])
                    xts[(nm, d)] = xt
            wo_t = []
            for dk in range(2):
                t = P.tile([128, D], bf, name=f"wo{dk}", tag=f"wo{dk}")
                nc.sync.dma_start(t, wot[dk * 128:(dk + 1) * 128, :])
                wo_t.append(t)

            bias_t = {}
            if has_bias:
                for nm, dram in (("bq", bq_d), ("bk", bk_d), ("bv", bv_d)):
                    for e in range(2):
                        t = P.tile([128, 1], f32, name=f"{nm}b{e}", tag=f"{nm}b{e}")
                        nc.sync.dma_start(
                            t, dram[e * 128:(e + 1) * 128].rearrange("(p o) -> p o", o=1)
                        )
                        bias_t[(nm, e)] = t

            # --- constants ---
            ident = P.tile([128, 128], bf, name="ident", tag="ident")
            make_identity(nc, ident)
            # utneg[j, k] = MASKNEG where j <= k else 0  (upper tri incl diag)
            # => (utneg.T @ I)[k, q] = utneg[q, k] = MASKNEG where q <= k
            # affine_select: out = in_ where (j - k > 0) else fill
            utneg = P.tile([128, 128], bf, name="utneg", tag="utneg")
            nc.gpsimd.memset(utneg, 0.0)
            nc.gpsimd.affine_select(
                out=utneg,
                in_=utneg,
                compare_op=mybir.AluOpType.is_gt,
                fill=MASKNEG,
                base=0,
                pattern=[[-1, 128]],
                channel_multiplier=1,
            )
            ones_row = P.tile([1, 512], bf, name="ones_row", tag="ones_row")
            nc.gpsimd.memset(ones_row, 1.0)
            # rank-1 blanket mask: negcol.T @ ones = MASKNEG everywhere
            negcol = P.tile([1, 128], bf, name="negcol", tag="negcol")
            nc.gpsimd.memset(negcol, MASKNEG)

            # --- persistent activations ---
            qhT = [P.tile([128, S], bf, name=f"qhT{e}", tag=f"qhT{e}") for e in range(2)]
            khT = [P.tile([128, S], bf, name=f"khT{e}", tag=f"khT{e}") for e in range(2)]
            vhT = [P.tile([128, S], bf, name=f"vhT{e}", tag=f"vhT{e}") for e in range(2)]
            vh = [P.tile([128, EG], bf, name=f"vh{st}", tag=f"vh{st}") for st in range(16)]
            ctxT = [P.tile([128, S], bf, name=f"ctxT{e}", tag=f"ctxT{e}") for e in range(2)]
            vspec = P.tile([1, EG], bf, name="vspec", tag="vspec")

            # ---------------- projections ----------------
            with tc.tile_pool(name="projpsum", bufs=8, space="PSUM") as PP:
                def project(nm, w_tiles, outT, bias_key):
                    psums = [
                        [PP.tile([128, 512], f32, name=f"pj{e}{sc}", tag="pj")
                         for sc in range(4)]
                        for e in range(2)
                    ]
                    for d in range(8):
                        xt = xts[(nm, d)]
                        for e in range(2):
                            for sc in range(4):
                                nc.tensor.matmul(
                                    psums[e][sc],
                                    lhsT=w_tiles[d][:, e * 128:(e + 1) * 128],
                                    rhs=xt[:, sc * 512:(sc + 1) * 512],
                                    start=(d == 0),
                                    stop=(d == 7),
                                )
                    for e in range(2):
                        for sc in range(4):
                            dst = outT[e][:, sc * 512:(sc + 1) * 512]
                            if has_bias:
                                nc.scalar.activation(
                                    dst, psums[e][sc], IDENT,
                                    bias=bias_t[(bias_key, e)], scale=1.0,
                                )
                            else:
                                nc.scalar.copy(dst, psums[e][sc])

                project("q", wq_t, qhT, "bq")
                project("k", wk_t, khT, "bk")
                project("v", wv_t, vhT, "bv")

            # vhT [256, S] -> vh [S, 256] via PE transpose
            with tc.tile_pool(name="tppsum", bufs=3, space="PSUM") as TP:
                for st in range(16):
                    for e in range(2):
                        tp = TP.tile([128, 128], bf, name="tp", tag="tp")
                        nc.tensor.transpose(
                            tp, vhT[e][:, st * 128:(st + 1) * 128], ident
                        )
                        nc.vector.tensor_copy(vh[st][:, e * 128:(e + 1) * 128], tp)

            # special row: vh[2047, :] / S at partition 0
            nc.sync.dma_start(vspec, vh[15][127:128, :])
            nc.vector.tensor_scalar_mul(vspec, vspec, 1.0 / S)

            # ---------------- attention (per head pair) ----------------
            with tc.tile_pool(name="attnpsum", bufs=1, space="PSUM") as AP_, \
                 tc.tile_pool(name="esb", bufs=10) as EP, \
                 tc.tile_pool(name="smalls", bufs=4) as SP:
                for p in range(2):
                    ctxps = [
                        AP_.tile([128, 512], f32, name=f"ctx{p}{qc}", tag="ctx", bufs=4)
                        for qc in range(4)
                    ]
                    Eprev = None
                    vhp_prev = None

                    def emit_ctx(kt, E_, vhp_):
                        qd_ = kt // 4
                        for qc in range(qd_, 4):
                            for a in range(2):
                                nc.tensor.matmul(
                                    ctxps[qc][64 * a:64 * (a + 1), :],
                                    lhsT=vhp_[:, 64 * a:64 * (a + 1)],
                                    rhs=E_[qc][:, 512 * a:512 * (a + 1)],
                                    start=(kt == 0),
                                    stop=False,
                                    skip_group_check=True,
                                )

                    for kt in range(16):
                        qd = kt // 4
                        j = kt % 4
                        nch = 4 - qd
                        E = [None] * 4
                        acc = SP.tile([128, 2, 4], f32, name="acc", tag="acc")
                        for qc in range(qd, 4):
                            # both heads' scores into one 2-bank psum tile
                            # (row-tiled K=64 matmuls run concurrently)
                            sc_ps = AP_.tile(
                                [128, 1024], f32, name="scps", tag="sc", bufs=2
                            )
                            diag = qc == qd
                            for a in range(2):
                                nc.tensor.matmul(
                                    sc_ps[:, 512 * a:512 * (a + 1)],
                                    lhsT=khT[p][64 * a:64 * (a + 1), kt * 128:(kt + 1) * 128],
                                    rhs=qhT[p][64 * a:64 * (a + 1), qc * 512:(qc + 1) * 512],
                                    start=True,
                                    stop=not diag,
                                    skip_group_check=diag,
                                )
                            if diag:
                                # accumulate -8e9 onto the q <= k region: the
                                # triangular pattern on the mixed 128-col block
                                # plus a rank-1 blanket on the columns left of
                                # it (q < k for the whole sub-block)
                                for a in range(2):
                                    nc.tensor.matmul(
                                        sc_ps[:, 512 * a + 128 * j:512 * a + 128 * (j + 1)],
                                        lhsT=utneg,
                                        rhs=ident,
                                        start=False,
                                        stop=(j == 0),
                                        skip_group_check=True,
                                    )
                                    if j > 0:
                                        nc.tensor.matmul(
                                            sc_ps[:, 512 * a:512 * a + 128 * j],
                                            lhsT=negcol,
                                            rhs=ones_row[0:1, 0:128 * j],
                                            start=False,
                                            stop=True,
                                            skip_group_check=True,
                                        )
                            et = EP.tile([128, 1024], bf, name="E", tag="E")
                            nc.scalar.activation(et, sc_ps, EXP, bias=0.0, scale=SCALE)
                            nc.vector.reduce_sum(
                                acc[:, :, qc - qd:qc - qd + 1],
                                et.rearrange("p (a x) -> p a x", a=2),
                                axis=mybir.AxisListType.X,
                            )
                            E[qc] = et
                        # row sums -> r, folded into vh'
                        ssum = SP.tile([128, 2], f32, name="ssum", tag="ssum")
                        nc.vector.reduce_sum(
                            ssum, acc[:, :, 0:nch], axis=mybir.AxisListType.X
                        )
                        if kt == 15:
                            # k=2047 row is fully masked (sum 0) — keep 1/sum
                            # finite; its uniform-1/S weights are added below
                            ssum2 = SP.tile([128, 2], f32, name="ssum2", tag="ssum2")
                            nc.vector.tensor_scalar_add(ssum2, ssum, 1.0e-30)
                            ssum = ssum2
                        rr = SP.tile([128, 2], f32, name="rr", tag="rr")
                        nc.vector.reciprocal(rr, ssum)
                        vhp = SP.tile([128, 128], bf, name="vhp", tag="vhp")
                        for a in range(2):
                            nc.vector.tensor_scalar_mul(
                                vhp[:, 64 * a:64 * (a + 1)],
                                vh[kt][:, 128 * p + 64 * a:128 * p + 64 * (a + 1)],
                                rr[:, a:a + 1],
                            )
                        # software pipeline: previous kt's ctx matmuls land
                        # after this kt's scores so the PE never waits on the
                        # exp/sum/reciprocal chain
                        if Eprev is not None:
                            emit_ctx(kt - 1, Eprev, vhp_prev)
                        Eprev, vhp_prev = E, vhp
                    emit_ctx(15, Eprev, vhp_prev)
                    # uniform contribution of the fully-masked k=S-1 row
                    for qc in range(4):
                        nc.tensor.matmul(
                            ctxps[qc],
                            lhsT=vspec[0:1, 128 * p:128 * (p + 1)],
                            rhs=ones_row,
                            start=False,
                            stop=True,
                            skip_group_check=True,
                        )
                        nc.vector.tensor_copy(ctxT[p][:, qc * 512:(qc + 1) * 512], ctxps[qc])

            # ---------------- output projection ----------------
            with tc.tile_pool(name="oppsum", bufs=4, space="PSUM") as OP, \
                 tc.tile_pool(name="osb", bufs=4) as OS:
                for st in range(16):
                    for oc in range(2):
                        ps = OP.tile([128, 512], f32, name="op_ps", tag="op")
                        for dk in range(2):
                            nc.tensor.matmul(
                                ps,
                                lhsT=ctxT[dk][:, st * 128:(st + 1) * 128],
                                rhs=wo_t[dk][:, oc * 512:(oc + 1) * 512],
                                start=(dk == 0),
                                stop=(dk == 1),
                            )
                        ob = OS.tile([128, 512], bf, name="ob", tag="ob")
                        if (st + oc) % 2 == 0:
                            nc.vector.tensor_copy(ob, ps)
                        else:
                            nc.scalar.copy(ob, ps)
                        nc.sync.dma_start(
                            out_d[st * 128:(st + 1) * 128, oc * 512:(oc + 1) * 512], ob
                        )

    nc.compile()
    return nc


_NC_CACHE = {}


def _get_nc(has_bias: bool):
    if has_bias not in _NC_CACHE:
        _NC_CACHE[has_bias] = _build_kernel(has_bias)
    return _NC_CACHE[has_bias]


def make_in_maps(q, k, v, Wq, bq, Wk, bk, Wv, bv, Wo, bo, has_bias):
    WqT = np.ascontiguousarray(Wq.T).astype(BF16)
    WkT = np.ascontiguousarray(Wk.T).astype(BF16)
    WvT = np.ascontiguousarray(Wv.T).astype(BF16)
    WoT = np.ascontiguousarray(Wo.T)
    xqt = [np.ascontiguousarray(q[b_].T).astype(BF16) for b_ in range(B)]
    xkt = [np.ascontiguousarray(k[b_].T).astype(BF16) for b_ in range(B)]
    xvt = [np.ascontiguousarray(v[b_].T).astype(BF16) for b_ in range(B)]
    in_maps = []
    for c in range(NCORES):
        b_, g = c // GROUPS, c % GROUPS
        sl = slice(g * EG, (g + 1) * EG)
        m = {
            "xqt": xqt[b_],
            "xkt": xkt[b_],
            "xvt": xvt[b_],
            "wqt": np.ascontiguousarray(WqT[:, sl]),
            "wkt": np.ascontiguousarray(WkT[:, sl]),
            "wvt": np.ascontiguousarray(WvT[:, sl]),
            "wot": np.ascontiguousarray(WoT[sl, :]).astype(BF16),
        }
        if has_bias:
            m["bq_s"] = np.ascontiguousarray(bq[sl]).astype(np.float32)
            m["bk_s"] = np.ascontiguousarray(bk[sl]).astype(np.float32)
            m["bv_s"] = np.ascontiguousarray(bv[sl]).astype(np.float32)
        in_maps.append(m)
    return in_maps


def gather(results, bo):
    out = np.zeros((B, S, D), np.float32)
    for b_ in range(B):
        acc = np.zeros((S, D), np.float32)
        for g in range(GROUPS):
            acc += results[b_ * GROUPS + g]["out"].astype(np.float32)
        out[b_] = acc + bo.astype(np.float32)[None, :]
    return out


def kernel(q, k, v, Wq, bq, Wk, bk, Wv, bv, Wo, bo, **run_kwargs):
    q, k, v = (np.asarray(x, np.float32) for x in (q, k, v))
    Wq, bq, Wk, bk, Wv, bv, Wo, bo = (
        np.asarray(x, np.float32) for x in (Wq, bq, Wk, bk, Wv, bv, Wo, bo)
    )
    has_bias = bool(
        max(np.abs(bq).max(), np.abs(bk).max(), np.abs(bv).max()) > 0
    )
    nc = _get_nc(has_bias)
    in_maps = make_in_maps(q, k, v, Wq, bq, Wk, bk, Wv, bv, Wo, bo, has_bias)
    res = run_bass_kernel_spmd(
        nc, in_maps, core_ids=list(range(NCORES)), **run_kwargs
    )
    out = gather(res.results, bo)
    if run_kwargs:
        return out, res
    return out
